# revision 28
# baseline (speedup 1.0000x reference)
"""CantorAttention Trainium2 kernel (8 NeuronCores, SPMD).

Strategy
--------
Shard (batch=2) x (head-pairs=4) across the 8 cores: core c handles batch
c//4 and heads {2*(c%4), 2*(c%4)+1}.  QKV projection is column-sharded,
output projection row-sharded per head pair; partial outputs are summed on
host.

The sparse gather `k[:, :, routes, :]` is turned into *dense band attention*
by a host-side permutation: sorting positions so that each query's K=64
routed keys fall in a small contiguous window (for the Cantor-route
structure, a 128-aligned window of <=3 x 128 keys per 128-query tile).
Duplicate / arbitrary routes are handled exactly via a per-(query,key)
count mask multiplied into exp(scores); unstructured routes degrade
gracefully to the full dense 2048-key window.

Perf notes vs the first-cut kernel:
  * all DRAM tensors are stored in SBUF layout ([128, N], contiguous per
    partition) so every DMA is a single fat 2D transfer (no 256B packets,
    ~0.6us descriptor gen instead of 10.8us for the mask rearrange)
  * stage A runs np-half-outer / contraction-chunk-inner so the first
    matmul only needs wqkv + the first xT chunk (~1.4MB, not 4.1MB)
  * warm-up transposes keep the PE HAM clock-gate busy during the input
    DMA so real matmuls run at 2.4GHz sooner
  * score jobs / PV / output projection / output DMA are interleaved with
    stage A and each other (the old kernel ran them as strict phases,
    leaving a 13.5us DMA-wait head and a 13.7us projection+DMA tail)
  * elementwise work is spread over Scalar/Vector/GpSimd
  * out-projection matmuls share each Wout chunk across two 512-col
    streams (LDWEIGHTS amortization), processing tile groups in pairs
"""

import numpy as np
import ml_dtypes

import concourse.bass as bass
import concourse.tile as tile
from concourse import bacc, mybir, masks
from concourse.bass_utils import run_bass_kernel_spmd

BF16 = ml_dtypes.bfloat16
B, S, DIM, H, HD, KNN = 2, 2048, 512, 8, 64, 64
NCORES = 8
T = 128           # queries per tile
NT = S // T       # 16 query tiles
NG = S // T       # 16 key chunks
SCALE = 1.0 / float(np.sqrt(HD))
CCH = DIM // 128  # 4 contraction chunks of the model dim
VSTR = 132        # v block stride: [v_h0 64 | ones 2 | v_h1 64 | ones 2]


# ----------------------------------------------------------------------------
# Host-side planning: permutation + per-tile key windows + count masks
# ----------------------------------------------------------------------------

def _cantor_perm() -> np.ndarray:
    """Sort order of positions by their Cantor-set coordinate (the structure
    the reference's routes are built from)."""
    x = np.arange(S, dtype=np.float64) / max(1, S - 1)
    x = np.clip(x, 1e-06, 1.0 - 1e-06)
    val = np.zeros(S, dtype=np.float64)
    factor = 0.5
    for _ in range(8):
        x *= 3.0
        digit = np.floor(x)
        x -= digit
        val += (digit == 2.0) * factor
        factor *= 0.5
    return np.argsort(val.astype(np.float32), kind="stable")


def _windows_for(perm: np.ndarray, routes: np.ndarray):
    inv = np.empty(S, np.int64)
    inv[perm] = np.arange(S)
    r_q = inv[routes][perm]  # (S, K): sorted-query -> sorted key positions
    lo = np.empty(NT, np.int64)
    nkc = np.empty(NT, np.int64)
    for t in range(NT):
        blk = r_q[t * T:(t + 1) * T]
        lo[t] = (blk.min() // T) * T
        nkc[t] = -(-(blk.max() + 1 - lo[t]) // T)
    return r_q, lo, nkc


class Plan:
    pass


def _plan(routes: np.ndarray) -> Plan:
    candidates = [
        _cantor_perm(),
        np.arange(S),
        np.argsort(routes.min(axis=1), kind="stable"),
        np.argsort(np.median(routes, axis=1), kind="stable"),
    ]
    best = None
    for perm in candidates:
        r_q, lo, nkc = _windows_for(perm, routes)
        cost = int(nkc.sum())
        if best is None or cost < best[0]:
            best = (cost, perm, r_q, lo, nkc)
    _, perm, r_q, lo, nkc = best

    def covers_of(lo, nkc):
        cover = [[] for _ in range(NG)]
        for t in range(NT):
            for kc in range(int(nkc[t])):
                cover[int(lo[t]) // T + kc].append(t)
        return cover

    cover = covers_of(lo, nkc)
    if any(ts != list(range(ts[0], ts[0] + len(ts))) for ts in cover if ts):
        # adversarial routes: windows interleave; use full dense windows
        lo = np.zeros(NT, np.int64)
        nkc = np.full(NT, NG, np.int64)
        cover = covers_of(lo, nkc)

    p = Plan()
    p.perm, p.lo, p.nkc = perm, lo, nkc

    # score jobs: (g, t0, nt) pieces with nt <= 4 (N <= 512)
    pieces = []
    for g in range(NG):
        ts = cover[g]
        if not ts:
            continue
        i = 0
        while i < len(ts):
            nt = min(4, len(ts) - i)
            pieces.append((g, ts[i], nt))
            i += nt

    # structured case: g-major emission, all P^T tiles held in SBUF (phased).
    # dense case: (t0, g)-major emission with interleaved PV to bound liveness.
    p.phased = len(pieces) <= 24
    if not p.phased:
        pieces.sort(key=lambda x: (x[1], x[0]))

    jobs = []            # (g, t0, nt, block_base)
    piece_of = {}        # (g, t) -> (job_idx, t0)
    nblocks = 0
    for g, t0, nt in pieces:
        jidx = len(jobs)
        jobs.append((g, t0, nt, nblocks))
        for t in range(t0, t0 + nt):
            piece_of[(g, t)] = (jidx, t0)
        nblocks += nt
    p.jobs, p.piece_of, p.nblocks = jobs, piece_of, nblocks

    # count masks, in job-block order: mask[key_in_chunk, query_in_tile]
    maskG = np.zeros((nblocks, T, T), np.float32)
    for g, t0, nt, base in jobs:
        for j, t in enumerate(range(t0, t0 + nt)):
            blk = r_q[t * T:(t + 1) * T]
            sel = (blk // T) == g
            w = (blk % T)[sel]
            q_idx = np.broadcast_to(np.arange(T)[:, None], blk.shape)[sel]
            np.add.at(maskG, (base + j, w, q_idx), 1.0)
    p.maskG = maskG.astype(BF16)
    return p


# ----------------------------------------------------------------------------
# Device program
# ----------------------------------------------------------------------------

def _build(p: Plan, with_qk_bias: bool):
    f32 = mybir.dt.float32
    bf16 = mybir.dt.bfloat16
    lo, nkc = p.lo, p.nkc
    jobs = p.jobs
    nc = bacc.Bacc("TRN2", target_bir_lowering=False, debug=False,
                   num_devices=NCORES)

    # All DRAM tensors are pre-packed on the host into SBUF layout:
    # [128 partitions, free], contiguous along free per partition.
    xT_d = nc.dram_tensor("xT", [128, CCH * S], bf16, kind="ExternalInput").ap()
    wqkv_d = nc.dram_tensor("wqkv", [128, CCH * 384], bf16,
                            kind="ExternalInput").ap()
    wout_d = nc.dram_tensor("wout", [128, DIM], bf16, kind="ExternalInput").ap()
    maskG_d = nc.dram_tensor("maskG", [128, p.nblocks * 128], bf16,
                             kind="ExternalInput").ap()
    if with_qk_bias:
        bqk_d = nc.dram_tensor("bqk", [256, 1], f32, kind="ExternalInput").ap()
    outT_d = nc.dram_tensor("outT", [128, CCH * S], bf16,
                            kind="ExternalOutput").ap()

    ptg_bufs = len(jobs) if p.phased else 20
    one_mask_sb = p.nblocks <= 64

    # ---- emission schedule -------------------------------------------------
    # job j is runnable after stage-A quarter q when its key chunk and its
    # query tiles are projected.  dense fallback keeps plan order (the
    # pt-tile ring relies on (t0, g)-major liveness) and runs after quarter 3.
    qjobs = [[] for _ in range(4)]
    for j, (g, t0, nt, _) in enumerate(jobs):
        if p.phased:
            q = max(g // 4, -(-(t0 + nt) // 4) - 1)
        else:
            q = 3
        qjobs[min(q, 3)].append(j)
    order = [j for ql in qjobs for j in ql]
    pos_of = {j: i for i, j in enumerate(order)}
    # pv_pos[t]: position in `order` of the last job tile t depends on
    pv_pos = {}
    for t in range(NT):
        pv_pos[t] = max(pos_of[p.piece_of[(int(lo[t]) // T + kc, t)][0]]
                        for kc in range(int(nkc[t])))

    with tile.TileContext(nc) as tc:
        with (
            tc.tile_pool(name="persist", bufs=1) as persist,
            tc.tile_pool(name="ps", bufs=6, space="PSUM") as psp,
            tc.tile_pool(name="ps_small", bufs=2, space="PSUM") as pss_small,
            tc.tile_pool(name="ptg", bufs=ptg_bufs) as ptgp,
            tc.tile_pool(name="maskst", bufs=16) as maskp,
            tc.tile_pool(name="attn", bufs=12) as attnp,
            tc.tile_pool(name="rz", bufs=4) as rzp,
            tc.tile_pool(name="atw", bufs=3) as atwp,
        ):
            xT = persist.tile([128, CCH * S], bf16, tag="xT")
            qkT = persist.tile([128, 2 * S], bf16, tag="qkT")
            vT = persist.tile([128, S], bf16, tag="vT")
            v_sb = persist.tile([128, NT * VSTR], bf16, tag="v")
            wqkv = persist.tile([128, CCH * 384], bf16, tag="wqkv")
            wout = persist.tile([128, DIM], bf16, tag="wout")
            outT = persist.tile([128, CCH * S], bf16, tag="outT")
            ident = persist.tile([128, 128], bf16, tag="ident")
            if one_mask_sb:
                maskA = persist.tile([128, p.nblocks * 128], bf16, tag="maskA")

            # ---- input DMAs (single fat transfers, compute-order) ----------
            nc.sync.dma_start(wqkv[:], wqkv_d)
            # mask slice boundaries (~3 slices so early jobs don't wait on
            # the whole mask; slice 0 is queued before the xT h1 halves)
            if one_mask_sb:
                nsl = 3
                cut = [0]
                for i in range(1, nsl):
                    jcut = (len(jobs) * i) // nsl
                    cut.append(jobs[jcut][3] if jcut < len(jobs) else p.nblocks)
                cut.append(p.nblocks)

                def mask_dma(i):
                    a, b = cut[i], cut[i + 1]
                    if b > a:
                        nc.sync.dma_start(maskA[:, a * 128:b * 128],
                                          maskG_d[:, a * 128:b * 128])
            # h0 halves first: stage-A quarters 0-1 only need cols [0,1024)
            # of each contraction chunk, so their deps land after ~1.4MB
            for c in range(CCH):
                o = c * S
                nc.sync.dma_start(xT[:, o:o + 1024], xT_d[:, o:o + 1024])
            if one_mask_sb:
                mask_dma(0)
            nc.sync.dma_start(wout[:], wout_d)
            for c in range(CCH):
                o = c * S + 1024
                nc.sync.dma_start(xT[:, o:o + 1024], xT_d[:, o:o + 1024])
            if with_qk_bias:
                bqk = persist.tile([128, 2], f32, tag="bqk")
                nc.sync.dma_start(
                    bqk[:].rearrange("p (c f) -> p c f", c=2),
                    bqk_d.rearrange("(c p) f -> p c f", p=128))
            if one_mask_sb:
                mask_dma(1)
                mask_dma(2)

            masks.make_identity(nc, ident[:])
            nc.vector.memset(
                v_sb[:].rearrange("p (g f) -> p g f", g=2 * NT)[:, :, 64:66],
                1.0)

            # ---- PE warm-up: keep HAM busy during the input DMA ------------
            def warmup(n):
                for _ in range(n):
                    psw = pss_small.tile([128, 128], bf16, tag="O",
                                         name="psw")
                    nc.tensor.transpose(psw[:], ident[:], ident[:])

            warmup(7)

            # ---- stage A quarter: qkv^T for sequence cols [o, o+512) -------
            # quarters (not halves) so score jobs can start after ~1/4 of the
            # projection: the element-engine-bound softmax pipeline overlaps
            # the PE-bound projection instead of running after it.
            def stage_a(qtr):
                o = qtr * 512
                pets = {}
                for f in (0, 1, 2):
                    pets[f] = psp.tile([128, 512], f32, tag="S2",
                                       name=f"stA{qtr}_{f}")
                for c in range(CCH):
                    for f in (1, 0, 2):   # k first so its drain starts first
                        nc.tensor.matmul(
                            pets[f][:],
                            lhsT=wqkv[:, c * 384 + f * 128:
                                      c * 384 + (f + 1) * 128],
                            rhs=xT[:, c * S + o:c * S + o + 512],
                            start=(c == 0), stop=(c == CCH - 1))
                    if qtr == 0 and c < 3:
                        # pad the PE pipe while the next xT chunk streams in
                        warmup((5, 4, 2)[c])
                if with_qk_bias:
                    nc.vector.tensor_scalar_add(
                        qkT[:, S + o:S + o + 512], pets[1][:], bqk[:, 1:2])
                    nc.vector.tensor_scalar_add(
                        qkT[:, o:o + 512], pets[0][:], bqk[:, 0:1])
                else:
                    nc.scalar.copy(qkT[:, S + o:S + o + 512], pets[1][:])
                    nc.vector.tensor_copy(qkT[:, o:o + 512], pets[0][:])
                if qtr % 2 == 0:
                    nc.vector.tensor_copy(vT[:, o:o + 512], pets[2][:])
                else:
                    nc.scalar.copy(vT[:, o:o + 512], pets[2][:])
                for g in range(4 * qtr, 4 * qtr + 4):
                    psv = pss_small.tile([128, 128], bf16, tag="O")
                    nc.tensor.transpose(psv[:], vT[:, g * 128:(g + 1) * 128],
                                        ident[:])
                    nc.vector.tensor_copy(
                        v_sb[:, g * VSTR:g * VSTR + VSTR].rearrange(
                            "p (h f) -> p h f", h=2)[:, :, 0:64],
                        psv[:].rearrange("p (h f) -> p h f", h=2))

            # ---- score job: S^T = k_g^T.T @ q^T, exp, count-mask -----------
            pt_tiles = {}
            attn_tiles = {}

            def emit_job(jidx, tail_job=False):
                g, t0, nt, base = jobs[jidx]
                nq = nt * 128
                if one_mask_sb:
                    mslice = maskA[:, base * 128:base * 128 + nq]
                else:
                    mt = maskp.tile([128, 512], bf16, tag="mask")
                    nc.sync.dma_start(mt[:, 0:nq],
                                      maskG_d[:, base * 128:base * 128 + nq])
                    mslice = mt[:, 0:nq]
                pss = [psp.tile([128, 512], f32, tag="S2",
                                name=f"pss{jidx}_{h}") for h in range(2)]
                for h in range(2):
                    hp = h * 64
                    nc.tensor.matmul(
                        pss[h][:, 0:nq],
                        lhsT=qkT[hp:hp + 64, S + g * 128:S + (g + 1) * 128],
                        rhs=qkT[hp:hp + 64, t0 * 128:t0 * 128 + nq],
                        start=True, stop=True)
                pt = ptgp.tile([128, 1024], bf16, tag="ptg")
                for h in range(2):
                    nc.scalar.activation(
                        pt[:, h * nq:(h + 1) * nq], pss[h][:, 0:nq],
                        mybir.ActivationFunctionType.Exp, scale=SCALE)
                nc.vector.tensor_mul(pt[:, 0:nq], pt[:, 0:nq], mslice)
                # gpsimd is ~3x slower per element: keep it off the critical
                # chain of the final jobs (nothing left to hide it behind)
                h1eng = nc.vector if tail_job else nc.gpsimd
                h1eng.tensor_mul(pt[:, nq:2 * nq], pt[:, nq:2 * nq], mslice)
                pt_tiles[jidx] = (pt, nq)

            # ---- PV + softmax normalize ------------------------------------
            def emit_pv(t):
                attn = attnp.tile([128, 128], bf16, tag="attn")
                attn_tiles[t] = attn
                nk = int(nkc[t])
                pso = pss_small.tile([128, 136], f32, tag="O")
                for h in range(2):
                    hb = h * 68
                    for kc in range(nk):
                        gg = int(lo[t]) // T + kc
                        jj, tt0 = p.piece_of[(gg, t)]
                        ptile, jnq = pt_tiles[jj]
                        coff = h * jnq + (t - tt0) * 128
                        nc.tensor.matmul(
                            pso[:, hb:hb + 65],
                            lhsT=ptile[:, coff:coff + 128],
                            rhs=v_sb[:, gg * VSTR + h * 66:
                                     gg * VSTR + h * 66 + 65],
                            start=(kc == 0), stop=(kc == nk - 1))
                rz = rzp.tile([128, 2], f32, tag="rz")
                pso3 = pso[:].rearrange("p (h f) -> p h f", h=2)
                rz3 = rz[:].rearrange("p (h f) -> p h f", h=2)
                nc.vector.reciprocal(rz3, pso3[:, :, 64:65])
                nc.vector.tensor_mul(
                    attn[:].rearrange("p (h f) -> p h f", h=2),
                    pso3[:, :, 0:64], rz3.broadcast_to([128, 2, 64]))

            # ---- attn^T staging (lagging each PV) --------------------------
            aTw_tiles = {}

            def emit_attn_t(t):
                tg = t // 4
                if tg not in aTw_tiles:
                    aTw_tiles[tg] = atwp.tile([128, 512], bf16, tag="aTw",
                                              name=f"aTw{tg}")
                aTw = aTw_tiles[tg]
                j = t % 4
                pst = pss_small.tile([128, 128], bf16, tag="O")
                nc.tensor.transpose(pst[:], attn_tiles[t][:], ident[:])
                if j % 2 == 0:
                    nc.scalar.copy(aTw[:, j * 128:(j + 1) * 128], pst[:])
                else:
                    nc.vector.tensor_copy(aTw[:, j * 128:(j + 1) * 128],
                                          pst[:])

            # ---- projection for one tile group (4 tiles, 512 queries) ------
            def emit_proj(tg):
                aTw = aTw_tiles[tg]
                for ob in range(2):
                    pp = [psp.tile([128, 512], f32, tag="S2",
                                   name=f"proj{tg}_{ob}_{k2}")
                          for k2 in range(2)]
                    for k2 in range(2):
                        oc = ob * 2 + k2
                        nc.tensor.matmul(
                            pp[k2][:],
                            lhsT=wout[:, oc * 128:(oc + 1) * 128],
                            rhs=aTw[:], start=True, stop=True)
                    for k2 in range(2):
                        oc = ob * 2 + k2
                        dst = outT[:, tg * 2048 + oc * 512:
                                   tg * 2048 + (oc + 1) * 512]
                        if k2 == 0:
                            nc.vector.tensor_copy(dst, pp[k2][:])
                        else:
                            nc.scalar.copy(dst, pp[k2][:])
                    o = tg * 2048 + ob * 1024
                    nc.sync.dma_start(outT_d[:, o:o + 1024],
                                      outT[:, o:o + 1024])

            # ---- interleaved emission --------------------------------------
            # attn^T transposes run 2 PVs behind so the PE never waits on the
            # softmax-normalize chain of the tile it just produced.
            pv_done = [False] * NT
            attn_t_queue = []
            attn_t_cnt = [0] * (NT // 4)
            tg_emitted = [False] * (NT // 4)

            def attn_t_one(t):
                emit_attn_t(t)
                tg = t // 4
                attn_t_cnt[tg] += 1
                if attn_t_cnt[tg] == 4 and not tg_emitted[tg]:
                    emit_proj(tg)
                    tg_emitted[tg] = True

            def flush_ready(max_pos, lag=2):
                for t in range(NT):
                    if not pv_done[t] and pv_pos[t] <= max_pos:
                        emit_pv(t)
                        pv_done[t] = True
                        attn_t_queue.append(t)
                        while len(attn_t_queue) > lag:
                            attn_t_one(attn_t_queue.pop(0))
                if lag == 0:
                    while attn_t_queue:
                        attn_t_one(attn_t_queue.pop(0))

            pos = 0
            for qtr in range(4):
                stage_a(qtr)
                for j in qjobs[qtr]:
                    emit_job(j, tail_job=(pos >= len(order) - 2))
                    flush_ready(pos - 2)
                    pos += 1
            flush_ready(len(order), lag=0)

    nc.compile()
    return nc


_CACHE = {}


def _get_program(p: Plan, with_qk_bias: bool):
    key = (tuple(int(v) for v in p.lo), tuple(int(v) for v in p.nkc),
           bool(with_qk_bias))
    if key not in _CACHE:
        _CACHE[key] = _build(p, with_qk_bias)
    return _CACHE[key]


# ----------------------------------------------------------------------------
# Entry point
# ----------------------------------------------------------------------------

def _pack_rows(a: np.ndarray) -> np.ndarray:
    """(128*c, n) -> [128, c*n] partition-packed SBUF layout."""
    c = a.shape[0] // 128
    return np.ascontiguousarray(
        a.reshape(c, 128, a.shape[1]).transpose(1, 0, 2).reshape(
            128, c * a.shape[1]))


def kernel(x, Wqkv, bqkv, Wout, bout, routes):
    x = np.asarray(x, np.float32)
    Wqkv = np.asarray(Wqkv, np.float32)
    bqkv = np.asarray(bqkv, np.float32)
    Wout = np.asarray(Wout, np.float32)
    bout = np.asarray(bout, np.float32)
    routes = np.asarray(routes)

    p = _plan(routes)
    perm = p.perm

    bq = bqkv[0:DIM]
    bk = bqkv[DIM:2 * DIM]
    bv = bqkv[2 * DIM:3 * DIM]
    with_qk_bias = bool(np.any(bq) or np.any(bk))

    nc = _get_program(p, with_qk_bias)

    # mask in SBUF layout: [128 key partitions, block*128 + query]
    maskT = np.ascontiguousarray(
        p.maskG.transpose(1, 0, 2).reshape(128, p.nblocks * 128))

    xT_packed = [None, None]
    for b in range(B):
        xT_packed[b] = _pack_rows(x[b].T[:, perm].astype(BF16))

    in_maps = []
    for c in range(NCORES):
        b = c // 4
        h0 = 2 * (c % 4)
        cols = slice(h0 * HD, (h0 + 2) * HD)
        wqkv = np.concatenate(
            [Wqkv[:, cols],
             Wqkv[:, DIM + h0 * HD:DIM + (h0 + 2) * HD],
             Wqkv[:, 2 * DIM + h0 * HD:2 * DIM + (h0 + 2) * HD]], axis=1)
        m = {
            "xT": xT_packed[b],
            "wqkv": _pack_rows(wqkv.astype(BF16)),
            "wout": np.ascontiguousarray(
                Wout[h0 * HD:(h0 + 2) * HD, :]).astype(BF16),
            "maskG": maskT,
        }
        if with_qk_bias:
            m["bqk"] = np.concatenate(
                [bq[h0 * HD:(h0 + 2) * HD],
                 bk[h0 * HD:(h0 + 2) * HD]]).reshape(256, 1).astype(np.float32)
        in_maps.append(m)

    global _last_in_maps
    _last_in_maps = in_maps
    res = run_bass_kernel_spmd(nc, in_maps, core_ids=list(range(NCORES)))

    out = np.zeros((B, S, DIM), np.float32)
    for c in range(NCORES):
        b = c // 4
        # outT: [128, tgrp*2048 + oc*512 + q] -> (S, DIM) permuted
        ot = res.results[c]["outT"].astype(np.float32)
        yT = ot.reshape(128, NT // 4, CCH, 512).transpose(
            2, 0, 1, 3).reshape(DIM, S)
        out[b][perm] += yT.T
    out += bout[None, None, :]
    if np.any(bv):
        out += (bv @ Wout)[None, None, :]
    return out


# revision 30
# speedup vs baseline: 1.0444x; 1.0444x over previous
"""CantorAttention Trainium2 kernel (8 NeuronCores, SPMD).

Strategy
--------
Shard (batch=2) x (head-pairs=4) across the 8 cores: core c handles batch
c//4 and heads {2*(c%4), 2*(c%4)+1}.  QKV projection is column-sharded,
output projection row-sharded per head pair; partial outputs are summed on
host.

The sparse gather `k[:, :, routes, :]` is turned into *dense band attention*
by a host-side permutation: sorting positions so that each query's K=64
routed keys fall in a small contiguous window (for the Cantor-route
structure, a 128-aligned window of <=3 x 128 keys per 128-query tile).
Duplicate / arbitrary routes are handled exactly via a per-(query,key)
count mask multiplied into exp(scores); unstructured routes degrade
gracefully to the full dense 2048-key window.

Perf notes vs the first-cut kernel:
  * all DRAM tensors are stored in SBUF layout ([128, N], contiguous per
    partition) so every DMA is a single fat 2D transfer (no 256B packets,
    ~0.6us descriptor gen instead of 10.8us for the mask rearrange)
  * stage A runs np-half-outer / contraction-chunk-inner so the first
    matmul only needs wqkv + the first xT chunk (~1.4MB, not 4.1MB)
  * warm-up transposes keep the PE HAM clock-gate busy during the input
    DMA so real matmuls run at 2.4GHz sooner
  * score jobs / PV / output projection / output DMA are interleaved with
    stage A and each other (the old kernel ran them as strict phases,
    leaving a 13.5us DMA-wait head and a 13.7us projection+DMA tail)
  * elementwise work is spread over Scalar/Vector/GpSimd
  * out-projection matmuls share each Wout chunk across two 512-col
    streams (LDWEIGHTS amortization), processing tile groups in pairs
"""

import numpy as np
import ml_dtypes

import concourse.bass as bass
import concourse.tile as tile
from concourse import bacc, mybir, masks
from concourse.bass_utils import run_bass_kernel_spmd

BF16 = ml_dtypes.bfloat16
B, S, DIM, H, HD, KNN = 2, 2048, 512, 8, 64, 64
NCORES = 8
T = 128           # queries per tile
NT = S // T       # 16 query tiles
NG = S // T       # 16 key chunks
SCALE = 1.0 / float(np.sqrt(HD))
CCH = DIM // 128  # 4 contraction chunks of the model dim
VSTR = 132        # v block stride: [v_h0 64 | ones 2 | v_h1 64 | ones 2]


# ----------------------------------------------------------------------------
# Host-side planning: permutation + per-tile key windows + count masks
# ----------------------------------------------------------------------------

def _cantor_perm() -> np.ndarray:
    """Sort order of positions by their Cantor-set coordinate (the structure
    the reference's routes are built from)."""
    x = np.arange(S, dtype=np.float64) / max(1, S - 1)
    x = np.clip(x, 1e-06, 1.0 - 1e-06)
    val = np.zeros(S, dtype=np.float64)
    factor = 0.5
    for _ in range(8):
        x *= 3.0
        digit = np.floor(x)
        x -= digit
        val += (digit == 2.0) * factor
        factor *= 0.5
    return np.argsort(val.astype(np.float32), kind="stable")


def _windows_for(perm: np.ndarray, routes: np.ndarray):
    inv = np.empty(S, np.int64)
    inv[perm] = np.arange(S)
    r_q = inv[routes][perm]  # (S, K): sorted-query -> sorted key positions
    lo = np.empty(NT, np.int64)
    nkc = np.empty(NT, np.int64)
    for t in range(NT):
        blk = r_q[t * T:(t + 1) * T]
        lo[t] = (blk.min() // T) * T
        nkc[t] = -(-(blk.max() + 1 - lo[t]) // T)
    return r_q, lo, nkc


class Plan:
    pass


def _plan(routes: np.ndarray) -> Plan:
    candidates = [
        _cantor_perm(),
        np.arange(S),
        np.argsort(routes.min(axis=1), kind="stable"),
        np.argsort(np.median(routes, axis=1), kind="stable"),
    ]
    best = None
    for perm in candidates:
        r_q, lo, nkc = _windows_for(perm, routes)
        cost = int(nkc.sum())
        if best is None or cost < best[0]:
            best = (cost, perm, r_q, lo, nkc)
    _, perm, r_q, lo, nkc = best

    def covers_of(lo, nkc):
        cover = [[] for _ in range(NG)]
        for t in range(NT):
            for kc in range(int(nkc[t])):
                cover[int(lo[t]) // T + kc].append(t)
        return cover

    cover = covers_of(lo, nkc)
    if any(ts != list(range(ts[0], ts[0] + len(ts))) for ts in cover if ts):
        # adversarial routes: windows interleave; use full dense windows
        lo = np.zeros(NT, np.int64)
        nkc = np.full(NT, NG, np.int64)
        cover = covers_of(lo, nkc)

    p = Plan()
    p.perm, p.lo, p.nkc = perm, lo, nkc

    # score jobs: (g, t0, nt) pieces with nt <= 4 (N <= 512)
    pieces = []
    for g in range(NG):
        ts = cover[g]
        if not ts:
            continue
        i = 0
        while i < len(ts):
            nt = min(4, len(ts) - i)
            pieces.append((g, ts[i], nt))
            i += nt

    # structured case: g-major emission, all P^T tiles held in SBUF (phased).
    # dense case: (t0, g)-major emission with interleaved PV to bound liveness.
    p.phased = len(pieces) <= 24
    if not p.phased:
        pieces.sort(key=lambda x: (x[1], x[0]))

    jobs = []            # (g, t0, nt, block_base)
    piece_of = {}        # (g, t) -> (job_idx, t0)
    nblocks = 0
    for g, t0, nt in pieces:
        jidx = len(jobs)
        jobs.append((g, t0, nt, nblocks))
        for t in range(t0, t0 + nt):
            piece_of[(g, t)] = (jidx, t0)
        nblocks += nt
    p.jobs, p.piece_of, p.nblocks = jobs, piece_of, nblocks

    # count masks, in job-block order: mask[key_in_chunk, query_in_tile]
    maskG = np.zeros((nblocks, T, T), np.float32)
    for g, t0, nt, base in jobs:
        for j, t in enumerate(range(t0, t0 + nt)):
            blk = r_q[t * T:(t + 1) * T]
            sel = (blk // T) == g
            w = (blk % T)[sel]
            q_idx = np.broadcast_to(np.arange(T)[:, None], blk.shape)[sel]
            np.add.at(maskG, (base + j, w, q_idx), 1.0)
    p.maskG = maskG.astype(BF16)

    # exact per-job query interval: outside it the count mask is all-zero,
    # so scores/exp/mask-mul shrink to [qlo, qhi) (the P^T edges are zeroed
    # explicitly so PV still reads valid zeros).  ~2x area cut for Cantor.
    p.jqint = []
    for g, t0, nt, base in jobs:
        anyq = (maskG[base:base + nt] > 0).any(axis=1).reshape(-1)
        idx = np.nonzero(anyq)[0]
        p.jqint.append((int(idx.min()), int(idx.max()) + 1))
    return p


# ----------------------------------------------------------------------------
# Device program
# ----------------------------------------------------------------------------

def _build(p: Plan, with_qk_bias: bool):
    f32 = mybir.dt.float32
    bf16 = mybir.dt.bfloat16
    lo, nkc = p.lo, p.nkc
    jobs = p.jobs
    nc = bacc.Bacc("TRN2", target_bir_lowering=False, debug=False,
                   num_devices=NCORES)

    # All DRAM tensors are pre-packed on the host into SBUF layout:
    # [128 partitions, free], contiguous along free per partition.
    xT_d = nc.dram_tensor("xT", [128, CCH * S], bf16, kind="ExternalInput").ap()
    wqkv_d = nc.dram_tensor("wqkv", [128, CCH * 384], bf16,
                            kind="ExternalInput").ap()
    wout_d = nc.dram_tensor("wout", [128, DIM], bf16, kind="ExternalInput").ap()
    maskG_d = nc.dram_tensor("maskG", [128, p.nblocks * 128], bf16,
                             kind="ExternalInput").ap()
    if with_qk_bias:
        bqk_d = nc.dram_tensor("bqk", [256, 1], f32, kind="ExternalInput").ap()
    outT_d = nc.dram_tensor("outT", [128, CCH * S], bf16,
                            kind="ExternalOutput").ap()

    ptg_bufs = len(jobs) if p.phased else 20
    one_mask_sb = p.nblocks <= 64

    # ---- emission schedule -------------------------------------------------
    # job j is runnable after stage-A quarter q when its key chunk and its
    # query tiles are projected.  dense fallback keeps plan order (the
    # pt-tile ring relies on (t0, g)-major liveness) and runs after quarter 3.
    qjobs = [[] for _ in range(4)]
    for j, (g, t0, nt, _) in enumerate(jobs):
        if p.phased:
            q = max(g // 4, -(-(t0 + nt) // 4) - 1)
        else:
            q = 3
        qjobs[min(q, 3)].append(j)
    order = [j for ql in qjobs for j in ql]
    pos_of = {j: i for i, j in enumerate(order)}
    # pv_pos[t]: position in `order` of the last job tile t depends on
    pv_pos = {}
    for t in range(NT):
        pv_pos[t] = max(pos_of[p.piece_of[(int(lo[t]) // T + kc, t)][0]]
                        for kc in range(int(nkc[t])))

    with tile.TileContext(nc) as tc:
        with (
            tc.tile_pool(name="persist", bufs=1) as persist,
            tc.tile_pool(name="ps", bufs=6, space="PSUM") as psp,
            tc.tile_pool(name="ps_small", bufs=2, space="PSUM") as pss_small,
            tc.tile_pool(name="ptg", bufs=ptg_bufs) as ptgp,
            tc.tile_pool(name="maskst", bufs=16) as maskp,
            tc.tile_pool(name="attn", bufs=12) as attnp,
            tc.tile_pool(name="rz", bufs=4) as rzp,
            tc.tile_pool(name="atw", bufs=3) as atwp,
        ):
            xT = persist.tile([128, CCH * S], bf16, tag="xT")
            qkT = persist.tile([128, 2 * S], bf16, tag="qkT")
            vT = persist.tile([128, S], bf16, tag="vT")
            v_sb = persist.tile([128, NT * VSTR], bf16, tag="v")
            wqkv = persist.tile([128, CCH * 384], bf16, tag="wqkv")
            wout = persist.tile([128, DIM], bf16, tag="wout")
            outT = persist.tile([128, CCH * S], bf16, tag="outT")
            ident = persist.tile([128, 128], bf16, tag="ident")
            if one_mask_sb:
                maskA = persist.tile([128, p.nblocks * 128], bf16, tag="maskA")

            # ---- input DMAs (single fat transfers, compute-order) ----------
            nc.sync.dma_start(wqkv[:], wqkv_d)
            # mask slice boundaries (~3 slices so early jobs don't wait on
            # the whole mask; slice 0 is queued before the xT h1 halves)
            if one_mask_sb:
                nsl = 3
                cut = [0]
                for i in range(1, nsl):
                    jcut = (len(jobs) * i) // nsl
                    cut.append(jobs[jcut][3] if jcut < len(jobs) else p.nblocks)
                cut.append(p.nblocks)

                def mask_dma(i):
                    a, b = cut[i], cut[i + 1]
                    if b > a:
                        nc.sync.dma_start(maskA[:, a * 128:b * 128],
                                          maskG_d[:, a * 128:b * 128])
            # h0 halves first: stage-A quarters 0-1 only need cols [0,1024)
            # of each contraction chunk, so their deps land after ~1.4MB
            for c in range(CCH):
                o = c * S
                nc.sync.dma_start(xT[:, o:o + 1024], xT_d[:, o:o + 1024])
            if one_mask_sb:
                mask_dma(0)
            nc.sync.dma_start(wout[:], wout_d)
            for c in range(CCH):
                o = c * S + 1024
                nc.sync.dma_start(xT[:, o:o + 1024], xT_d[:, o:o + 1024])
            if with_qk_bias:
                bqk = persist.tile([128, 2], f32, tag="bqk")
                nc.sync.dma_start(
                    bqk[:].rearrange("p (c f) -> p c f", c=2),
                    bqk_d.rearrange("(c p) f -> p c f", p=128))
            if one_mask_sb:
                mask_dma(1)
                mask_dma(2)

            masks.make_identity(nc, ident[:])
            nc.vector.memset(
                v_sb[:].rearrange("p (g f) -> p g f", g=2 * NT)[:, :, 64:66],
                1.0)

            # ---- PE warm-up: keep HAM busy during the input DMA ------------
            def warmup(n):
                for _ in range(n):
                    psw = pss_small.tile([128, 128], bf16, tag="O",
                                         name="psw")
                    nc.tensor.transpose(psw[:], ident[:], ident[:])

            warmup(7)

            # ---- stage A quarter: qkv^T for sequence cols [o, o+512) -------
            # quarters (not halves) so score jobs can start after ~1/4 of the
            # projection: the element-engine-bound softmax pipeline overlaps
            # the PE-bound projection instead of running after it.
            def stage_a(qtr):
                o = qtr * 512
                pets = {}
                for f in (0, 1, 2):
                    pets[f] = psp.tile([128, 512], f32, tag="S2",
                                       name=f"stA{qtr}_{f}")
                for c in range(CCH):
                    for f in (1, 0, 2):   # k first so its drain starts first
                        nc.tensor.matmul(
                            pets[f][:],
                            lhsT=wqkv[:, c * 384 + f * 128:
                                      c * 384 + (f + 1) * 128],
                            rhs=xT[:, c * S + o:c * S + o + 512],
                            start=(c == 0), stop=(c == CCH - 1))
                    if qtr == 0 and c < 3:
                        # pad the PE pipe while the next xT chunk streams in
                        warmup((5, 4, 2)[c])
                if with_qk_bias:
                    nc.vector.tensor_scalar_add(
                        qkT[:, S + o:S + o + 512], pets[1][:], bqk[:, 1:2])
                    nc.vector.tensor_scalar_add(
                        qkT[:, o:o + 512], pets[0][:], bqk[:, 0:1])
                else:
                    nc.scalar.copy(qkT[:, S + o:S + o + 512], pets[1][:])
                    nc.vector.tensor_copy(qkT[:, o:o + 512], pets[0][:])
                if qtr % 2 == 0:
                    nc.vector.tensor_copy(vT[:, o:o + 512], pets[2][:])
                else:
                    nc.scalar.copy(vT[:, o:o + 512], pets[2][:])
                for g in range(4 * qtr, 4 * qtr + 4):
                    psv = pss_small.tile([128, 128], bf16, tag="O")
                    nc.tensor.transpose(psv[:], vT[:, g * 128:(g + 1) * 128],
                                        ident[:])
                    nc.vector.tensor_copy(
                        v_sb[:, g * VSTR:g * VSTR + VSTR].rearrange(
                            "p (h f) -> p h f", h=2)[:, :, 0:64],
                        psv[:].rearrange("p (h f) -> p h f", h=2))

            # ---- score job: S^T = k_g^T.T @ q^T, exp, count-mask -----------
            pt_tiles = {}
            attn_tiles = {}

            def emit_job(jidx, tail_job=False):
                g, t0, nt, base = jobs[jidx]
                nq = nt * 128
                if one_mask_sb:
                    mslice = maskA[:, base * 128:base * 128 + nq]
                else:
                    mt = maskp.tile([128, 512], bf16, tag="mask")
                    nc.sync.dma_start(mt[:, 0:nq],
                                      maskG_d[:, base * 128:base * 128 + nq])
                    mslice = mt[:, 0:nq]
                qlo, qhi = p.jqint[jidx]
                nqe = qhi - qlo
                pss = [psp.tile([128, 512], f32, tag="S2",
                                name=f"pss{jidx}_{h}") for h in range(2)]
                for h in range(2):
                    hp = h * 64
                    nc.tensor.matmul(
                        pss[h][:, qlo:qhi],
                        lhsT=qkT[hp:hp + 64, S + g * 128:S + (g + 1) * 128],
                        rhs=qkT[hp:hp + 64,
                                t0 * 128 + qlo:t0 * 128 + qhi],
                        start=True, stop=True)
                pt = ptgp.tile([128, 1024], bf16, tag="ptg")
                pt3 = pt[:, 0:2 * nq].rearrange("p (h f) -> p h f", h=2)
                if qlo > 0:
                    nc.vector.memset(pt3[:, :, 0:qlo], 0.0)
                if qhi < nq:
                    nc.gpsimd.memset(pt3[:, :, qhi:nq], 0.0)
                for h in range(2):
                    nc.scalar.activation(
                        pt[:, h * nq + qlo:h * nq + qhi], pss[h][:, qlo:qhi],
                        mybir.ActivationFunctionType.Exp, scale=SCALE)
                me = mslice[:, qlo:qhi]
                nc.vector.tensor_mul(pt[:, qlo:qhi], pt[:, qlo:qhi], me)
                # gpsimd is ~3x slower per element: keep it off the critical
                # chain of the final jobs (nothing left to hide it behind)
                h1eng = nc.vector if tail_job else nc.gpsimd
                h1eng.tensor_mul(pt[:, nq + qlo:nq + qhi],
                                 pt[:, nq + qlo:nq + qhi], me)
                pt_tiles[jidx] = (pt, nq)

            # ---- PV + softmax normalize ------------------------------------
            def emit_pv(t):
                attn = attnp.tile([128, 128], bf16, tag="attn")
                attn_tiles[t] = attn
                nk = int(nkc[t])
                pso = pss_small.tile([128, 136], f32, tag="O")
                for h in range(2):
                    hb = h * 68
                    for kc in range(nk):
                        gg = int(lo[t]) // T + kc
                        jj, tt0 = p.piece_of[(gg, t)]
                        ptile, jnq = pt_tiles[jj]
                        coff = h * jnq + (t - tt0) * 128
                        nc.tensor.matmul(
                            pso[:, hb:hb + 65],
                            lhsT=ptile[:, coff:coff + 128],
                            rhs=v_sb[:, gg * VSTR + h * 66:
                                     gg * VSTR + h * 66 + 65],
                            start=(kc == 0), stop=(kc == nk - 1))
                rz = rzp.tile([128, 2], f32, tag="rz")
                pso3 = pso[:].rearrange("p (h f) -> p h f", h=2)
                rz3 = rz[:].rearrange("p (h f) -> p h f", h=2)
                nc.vector.reciprocal(rz3, pso3[:, :, 64:65])
                nc.vector.tensor_mul(
                    attn[:].rearrange("p (h f) -> p h f", h=2),
                    pso3[:, :, 0:64], rz3.broadcast_to([128, 2, 64]))

            # ---- attn^T staging (lagging each PV) --------------------------
            aTw_tiles = {}

            def emit_attn_t(t):
                tg = t // 4
                if tg not in aTw_tiles:
                    aTw_tiles[tg] = atwp.tile([128, 512], bf16, tag="aTw",
                                              name=f"aTw{tg}")
                aTw = aTw_tiles[tg]
                j = t % 4
                pst = pss_small.tile([128, 128], bf16, tag="O")
                nc.tensor.transpose(pst[:], attn_tiles[t][:], ident[:])
                if j % 2 == 0:
                    nc.scalar.copy(aTw[:, j * 128:(j + 1) * 128], pst[:])
                else:
                    nc.vector.tensor_copy(aTw[:, j * 128:(j + 1) * 128],
                                          pst[:])

            # ---- projection for one tile group (4 tiles, 512 queries) ------
            def emit_proj(tg):
                aTw = aTw_tiles[tg]
                for ob in range(2):
                    pp = [psp.tile([128, 512], f32, tag="S2",
                                   name=f"proj{tg}_{ob}_{k2}")
                          for k2 in range(2)]
                    for k2 in range(2):
                        oc = ob * 2 + k2
                        nc.tensor.matmul(
                            pp[k2][:],
                            lhsT=wout[:, oc * 128:(oc + 1) * 128],
                            rhs=aTw[:], start=True, stop=True)
                    for k2 in range(2):
                        oc = ob * 2 + k2
                        dst = outT[:, tg * 2048 + oc * 512:
                                   tg * 2048 + (oc + 1) * 512]
                        if k2 == 0:
                            nc.vector.tensor_copy(dst, pp[k2][:])
                        else:
                            nc.scalar.copy(dst, pp[k2][:])
                    o = tg * 2048 + ob * 1024
                    nc.sync.dma_start(outT_d[:, o:o + 1024],
                                      outT[:, o:o + 1024])

            # ---- interleaved emission --------------------------------------
            # attn^T transposes run 2 PVs behind so the PE never waits on the
            # softmax-normalize chain of the tile it just produced.
            pv_done = [False] * NT
            attn_t_queue = []
            attn_t_cnt = [0] * (NT // 4)
            tg_emitted = [False] * (NT // 4)

            def attn_t_one(t):
                emit_attn_t(t)
                tg = t // 4
                attn_t_cnt[tg] += 1
                if attn_t_cnt[tg] == 4 and not tg_emitted[tg]:
                    emit_proj(tg)
                    tg_emitted[tg] = True

            def flush_ready(max_pos, lag=2):
                for t in range(NT):
                    if not pv_done[t] and pv_pos[t] <= max_pos:
                        emit_pv(t)
                        pv_done[t] = True
                        attn_t_queue.append(t)
                        while len(attn_t_queue) > lag:
                            attn_t_one(attn_t_queue.pop(0))
                if lag == 0:
                    while attn_t_queue:
                        attn_t_one(attn_t_queue.pop(0))

            pos = 0
            for qtr in range(4):
                stage_a(qtr)
                for j in qjobs[qtr]:
                    emit_job(j, tail_job=(pos >= len(order) - 2))
                    flush_ready(pos - 2)
                    pos += 1
            flush_ready(len(order), lag=0)

    nc.compile()
    return nc


_CACHE = {}


def _get_program(p: Plan, with_qk_bias: bool):
    key = (tuple(int(v) for v in p.lo), tuple(int(v) for v in p.nkc),
           bool(with_qk_bias))
    if key not in _CACHE:
        _CACHE[key] = _build(p, with_qk_bias)
    return _CACHE[key]


# ----------------------------------------------------------------------------
# Entry point
# ----------------------------------------------------------------------------

def _pack_rows(a: np.ndarray) -> np.ndarray:
    """(128*c, n) -> [128, c*n] partition-packed SBUF layout."""
    c = a.shape[0] // 128
    return np.ascontiguousarray(
        a.reshape(c, 128, a.shape[1]).transpose(1, 0, 2).reshape(
            128, c * a.shape[1]))


def kernel(x, Wqkv, bqkv, Wout, bout, routes):
    x = np.asarray(x, np.float32)
    Wqkv = np.asarray(Wqkv, np.float32)
    bqkv = np.asarray(bqkv, np.float32)
    Wout = np.asarray(Wout, np.float32)
    bout = np.asarray(bout, np.float32)
    routes = np.asarray(routes)

    p = _plan(routes)
    perm = p.perm

    bq = bqkv[0:DIM]
    bk = bqkv[DIM:2 * DIM]
    bv = bqkv[2 * DIM:3 * DIM]
    with_qk_bias = bool(np.any(bq) or np.any(bk))

    nc = _get_program(p, with_qk_bias)

    # mask in SBUF layout: [128 key partitions, block*128 + query]
    maskT = np.ascontiguousarray(
        p.maskG.transpose(1, 0, 2).reshape(128, p.nblocks * 128))

    xT_packed = [None, None]
    for b in range(B):
        xT_packed[b] = _pack_rows(x[b].T[:, perm].astype(BF16))

    in_maps = []
    for c in range(NCORES):
        b = c // 4
        h0 = 2 * (c % 4)
        cols = slice(h0 * HD, (h0 + 2) * HD)
        wqkv = np.concatenate(
            [Wqkv[:, cols],
             Wqkv[:, DIM + h0 * HD:DIM + (h0 + 2) * HD],
             Wqkv[:, 2 * DIM + h0 * HD:2 * DIM + (h0 + 2) * HD]], axis=1)
        m = {
            "xT": xT_packed[b],
            "wqkv": _pack_rows(wqkv.astype(BF16)),
            "wout": np.ascontiguousarray(
                Wout[h0 * HD:(h0 + 2) * HD, :]).astype(BF16),
            "maskG": maskT,
        }
        if with_qk_bias:
            m["bqk"] = np.concatenate(
                [bq[h0 * HD:(h0 + 2) * HD],
                 bk[h0 * HD:(h0 + 2) * HD]]).reshape(256, 1).astype(np.float32)
        in_maps.append(m)

    global _last_in_maps
    _last_in_maps = in_maps
    res = run_bass_kernel_spmd(nc, in_maps, core_ids=list(range(NCORES)))

    out = np.zeros((B, S, DIM), np.float32)
    for c in range(NCORES):
        b = c // 4
        # outT: [128, tgrp*2048 + oc*512 + q] -> (S, DIM) permuted
        ot = res.results[c]["outT"].astype(np.float32)
        yT = ot.reshape(128, NT // 4, CCH, 512).transpose(
            2, 0, 1, 3).reshape(DIM, S)
        out[b][perm] += yT.T
    out += bout[None, None, :]
    if np.any(bv):
        out += (bv @ Wout)[None, None, :]
    return out
